# revision 1
# baseline (speedup 1.0000x reference)
"""Trainium2 Bass kernel for ChromophoreSolventGNN (2x GCNConv + BN + mean-pool + MLP head).

Strategy (8 NeuronCores, SPMD):
  - Destination-shard nodes: core c owns contiguous node range [c*2560, (c+1)*2560)
    (N=20000 padded to 20480). Edges (incl. self-loops) are routed to the owner
    of their destination (col), sorted by destination block (128 nodes).
  - GCN aggregation is linear, so aggregate FIRST then project: (S @ X) W == S @ (X W).
  - Gather of source rows via dma_gather (HBM -> SBUF, 256B rows).
  - Scatter-add via one-hot matmul: per 128-edge chunk, lhsT = onehot[e, dst]
    (DVE: iota==col_local, scaled by edge norm), rhs = gathered rows; PSUM
    accumulates per destination block. fp16 operands, fp32 accumulate.
  - BatchNorm is shift-invariant => conv biases b1/b2 and fc1 bias drop out.
    Stats are per-feature partial sums AllReduce'd across cores.
  - After layer 1, per-core h1 slabs are AllGather'd for the layer-2 gather.
    Mean-pool via one-hot matmul with 1/count folded in; pooled sums
    AllReduce'd; the small MLP head is computed replicated on every core.
"""

import numpy as np

import concourse.bass as bass
import concourse.mybir as mybir
from concourse import bacc
from concourse.bass_utils import run_bass_kernel_spmd
from concourse.tile import TileContext

F32 = mybir.dt.float32
F16 = mybir.dt.float16
I16 = mybir.dt.int16

W = 8            # cores
N = 20000        # nodes
E = 320000       # edges
G = 512          # graphs
F_IN = 64
H1 = 128
H2 = 256
SOLV = 128
EPS = 1e-5

NB = 20                  # destination blocks of 128 nodes per core
PC = NB * 128            # nodes per core (2560)
NP = W * PC              # padded node count (20480)
CCH_MAX = 8              # chunks per gather call (<=1024 idxs, HW desc-ring limit)


def _bn_apply_params(nc, tl, st, colw, n_count, g_sb, be_sb, name):
    """From (sum, sumsq) slices compute per-partition scale/shift tiles."""
    mu = tl.tile([128, 1], F32, tag=f"mu{name}")
    nc.vector.tensor_scalar_mul(mu[:], st[:, colw : colw + 1], 1.0 / n_count)
    var = tl.tile([128, 1], F32, tag=f"var{name}")
    nc.vector.tensor_scalar_mul(var[:], st[:, colw + 1 : colw + 2], 1.0 / n_count)
    musq = tl.tile([128, 1], F32, tag=f"musq{name}")
    nc.vector.tensor_tensor(out=musq[:], in0=mu[:], in1=mu[:], op=mybir.AluOpType.mult)
    nc.vector.tensor_tensor(out=var[:], in0=var[:], in1=musq[:], op=mybir.AluOpType.subtract)
    nc.vector.tensor_scalar_add(var[:], var[:], EPS)
    rv = tl.tile([128, 1], F32, tag=f"rv{name}")
    nc.vector.reciprocal(out=rv[:], in_=var[:])
    rstd = tl.tile([128, 1], F32, tag=f"rstd{name}")
    nc.scalar.sqrt(out=rstd[:], in_=rv[:])
    sc = tl.tile([128, 1], F32, tag=f"sc{name}")
    nc.vector.tensor_tensor(out=sc[:], in0=g_sb[:], in1=rstd[:], op=mybir.AluOpType.mult)
    sh = tl.tile([128, 1], F32, tag=f"sh{name}")
    nc.vector.tensor_tensor(out=sh[:], in0=mu[:], in1=sc[:], op=mybir.AluOpType.mult)
    nc.vector.tensor_tensor(out=sh[:], in0=be_sb[:], in1=sh[:], op=mybir.AluOpType.subtract)
    return sc, sh


def _build_program(C, stop=99):
    """Build the SPMD Bass program. C = chunks (of 128 edge slots) per dst block.
    stop: debug aid - phases above this are omitted."""
    NC = NB * C           # chunks per core
    NE = NC * 128         # edge slots per core

    nc = bacc.Bacc("TRN2", target_bir_lowering=False, debug=False, num_devices=W)

    # ---- external inputs -------------------------------------------------
    x_d = nc.dram_tensor("x", [N, F_IN], F32, kind="ExternalInput")
    gidx_d = nc.dram_tensor("gidx", [128, NE // 16], I16, kind="ExternalInput")
    col_d = nc.dram_tensor("colloc", [128, NC], F32, kind="ExternalInput")
    nrm_d = nc.dram_tensor("enorm", [128, NC], F32, kind="ExternalInput")
    bg_d = nc.dram_tensor("batchg", [128, NB], F32, kind="ExternalInput")
    bw_d = nc.dram_tensor("bw", [128, NB], F32, kind="ExternalInput")
    iota_d = nc.dram_tensor("iota128", [128, 128], F16, kind="ExternalInput")
    iota5_d = nc.dram_tensor("iota512", [128, G], F16, kind="ExternalInput")
    id16_d = nc.dram_tensor("ident16", [128, 128], F16, kind="ExternalInput")
    id32_d = nc.dram_tensor("ident32", [128, 128], F32, kind="ExternalInput")
    sfT_d = nc.dram_tensor("sfT", [SOLV, G], F32, kind="ExternalInput")
    w1_d = nc.dram_tensor("w1", [F_IN, H1], F32, kind="ExternalInput")
    w2_d = nc.dram_tensor("w2", [H1, H2], F32, kind="ExternalInput")
    ws_d = nc.dram_tensor("ws", [SOLV, 128], F32, kind="ExternalInput")
    wf1_d = nc.dram_tensor("wf1", [128, 3, 128], F32, kind="ExternalInput")
    wf2_d = nc.dram_tensor("wf2", [128, 1], F32, kind="ExternalInput")
    g1_d = nc.dram_tensor("g1", [128, 1], F32, kind="ExternalInput")
    be1_d = nc.dram_tensor("be1", [128, 1], F32, kind="ExternalInput")
    g2_d = nc.dram_tensor("g2", [128, 2], F32, kind="ExternalInput")
    be2_d = nc.dram_tensor("be2", [128, 2], F32, kind="ExternalInput")
    gf1_d = nc.dram_tensor("gf1", [128, 1], F32, kind="ExternalInput")
    bef1_d = nc.dram_tensor("bef1", [128, 1], F32, kind="ExternalInput")
    bs_d = nc.dram_tensor("bs", [128, 1], F32, kind="ExternalInput")
    bf2_d = nc.dram_tensor("bf2", [1, 1], F32, kind="ExternalInput")

    out_d = nc.dram_tensor("out", [G, 1], F32, kind="ExternalOutput")

    # ---- internal DRAM ---------------------------------------------------
    h1loc_d = nc.dram_tensor("h1loc", [PC, H1], F16)
    h1full_d = nc.dram_tensor("h1full", [NP, H1], F16, addr_space="Shared")
    bn1i_d = nc.dram_tensor("bn1i", [128, 2], F32)
    bn1o_d = nc.dram_tensor("bn1o", [128, 2], F32, addr_space="Shared")
    bn2i_d = nc.dram_tensor("bn2i", [128, 4], F32)
    bn2o_d = nc.dram_tensor("bn2o", [128, 4], F32, addr_space="Shared")
    pli_d = nc.dram_tensor("pli", [2 * 128, G], F32)
    plo_d = nc.dram_tensor("plo", [2 * 128, G], F32, addr_space="Shared")

    RG = [list(range(W))]
    CCH = CCH_MAX
    while NC % CCH:
        CCH -= 1
    NGC = NC // CCH        # gather calls per layer
    CPC = CCH * 128        # edge slots per gather call (<=1024)
    NKC = PC // 512        # 512-node column chunks

    with TileContext(nc) as tc:
        with tc.tile_pool(name="const", bufs=1) as cst, \
             tc.tile_pool(name="ohpool", bufs=1) as ohp, \
             tc.tile_pool(name="ps", bufs=2, space="PSUM") as ps, \
             tc.tile_pool(name="psacc", bufs=3, space="PSUM") as psacc:

            # ---------- setup: constants ----------
            def load_const(name, dram, shape, dt):
                t = cst.tile(shape, dt, name=name)
                nc.sync.dma_start(out=t[:], in_=dram[:])
                return t

            gidx_sb = load_const("gidx_sb", gidx_d, [128, NE // 16], I16)
            col_sb = load_const("col_sb", col_d, [128, NC], F32)
            nrm_sb = load_const("nrm_sb", nrm_d, [128, NC], F32)
            bg_sb = load_const("bg_sb", bg_d, [128, NB], F32)
            bw_sb = load_const("bw_sb", bw_d, [128, NB], F32)
            iota_sb = load_const("iota_sb", iota_d, [128, 128], F16)
            iota5_sb = load_const("iota5_sb", iota5_d, [128, G], F16)
            id16_sb = load_const("id16_sb", id16_d, [128, 128], F16)
            id32_sb = load_const("id32_sb", id32_d, [128, 128], F32)
            sfT_sb = load_const("sfT_sb", sfT_d, [SOLV, G], F32)
            w1_sb = load_const("w1_sb", w1_d, [F_IN, H1], F32)
            w2_sb = load_const("w2_sb", w2_d, [H1, H2], F32)
            ws_sb = load_const("ws_sb", ws_d, [SOLV, 128], F32)
            wf1_sb = load_const("wf1_sb", wf1_d, [128, 3, 128], F32)
            wf2_sb = load_const("wf2_sb", wf2_d, [128, 1], F32)
            g1_sb = load_const("g1_sb", g1_d, [128, 1], F32)
            be1_sb = load_const("be1_sb", be1_d, [128, 1], F32)
            g2_sb = load_const("g2_sb", g2_d, [128, 2], F32)
            be2_sb = load_const("be2_sb", be2_d, [128, 2], F32)
            gf1_sb = load_const("gf1_sb", gf1_d, [128, 1], F32)
            bef1_sb = load_const("bef1_sb", bef1_d, [128, 1], F32)
            bs_sb = load_const("bs_sb", bs_d, [128, 1], F32)
            bf2_sb = load_const("bf2_sb", bf2_d, [1, 1], F32)

            # ---------- one-hot generation (shared by both layers) ----------
            oh = ohp.tile([128, NC, 128], F16)
            for j in range(NC):
                nc.vector.tensor_scalar(
                    out=oh[:, j, :],
                    in0=iota_sb[:],
                    scalar1=col_sb[:, j : j + 1],
                    scalar2=nrm_sb[:, j : j + 1],
                    op0=mybir.AluOpType.is_equal,
                    op1=mybir.AluOpType.mult,
                )

            # ---------- layer 1 ----------
            l1 = tc.alloc_tile_pool(name="l1", bufs=1)
            l1s = tc.alloc_tile_pool(name="l1s", bufs=3)
            agg1_sb = l1.tile([128, NB, F_IN], F32)
            psum_map = {}
            for gcall in range(NGC):
                xg = l1s.tile([128, CCH, F_IN], F32, tag="xg")
                nc.gpsimd.dma_gather(
                    out_ap=xg[:],
                    in_ap=x_d[:],
                    idxs_ap=gidx_sb[:, gcall * (CPC // 16) : (gcall + 1) * (CPC // 16)],
                    num_idxs=CPC,
                    num_idxs_reg=CPC,
                    elem_size=F_IN,
                )
                xg16 = l1s.tile([128, CCH, F_IN], F16, tag="xg16")
                nc.scalar.copy(out=xg16[:], in_=xg[:])
                for jj in range(CCH):
                    j = gcall * CCH + jj
                    b, cidx = divmod(j, C)
                    if cidx == 0:
                        acc1 = psacc.tile([128, F_IN], F32, tag="acc")
                        psum_map[b] = acc1
                    nc.tensor.matmul(
                        out=psum_map[b][:],
                        lhsT=oh[:, j, :],
                        rhs=xg16[:, jj, :],
                        start=(cidx == 0),
                        stop=(cidx == C - 1),
                    )
                    if cidx == C - 1:
                        nc.vector.tensor_copy(out=agg1_sb[:, b, :], in_=psum_map[b][:])

            if stop >= 2:
                # transpose agg1 -> [F_IN, PC] feature-major
                aggxT = l1.tile([F_IN, NB, 128], F32)
                for b in range(NB):
                    pt = ps.tile([F_IN, 128], F32, tag="pt")
                    nc.tensor.transpose(out=pt[:], in_=agg1_sb[:, b, :], identity=id32_sb[:])
                    nc.vector.tensor_copy(out=aggxT[:, b, :], in_=pt[:])

                # project: h1T [H1, PC] = W1.T @ aggxT ; BN1 stats along nodes
                h1T_sb = l1.tile([H1, NB * 128], F32)
                s1p = l1.tile([128, 16], F32)
                aggxT_f = aggxT[:].rearrange("p b n -> p (b n)")
                for k in range(NKC):
                    ph = ps.tile([H1, 512], F32, tag="ph")
                    nc.tensor.matmul(
                        out=ph[:], lhsT=w1_sb[:],
                        rhs=aggxT_f[:, k * 512 : (k + 1) * 512],
                        start=True, stop=True,
                    )
                    nc.vector.tensor_copy(out=h1T_sb[:, k * 512 : (k + 1) * 512], in_=ph[:])
                    nc.vector.tensor_reduce(
                        out=s1p[:, k : k + 1], in_=ph[:],
                        axis=mybir.AxisListType.X, op=mybir.AluOpType.add,
                    )
                    sq = l1s.tile([H1, 512], F32, tag="sq1")
                    nc.scalar.square(out=sq[:], in_=ph[:])
                    nc.vector.tensor_reduce(
                        out=s1p[:, NKC + k : NKC + k + 1], in_=sq[:],
                        axis=mybir.AxisListType.X, op=mybir.AluOpType.add,
                    )
                st1 = l1.tile([128, 2], F32)
                nc.vector.tensor_reduce(out=st1[:, 0:1], in_=s1p[:, 0:NKC],
                                        axis=mybir.AxisListType.X, op=mybir.AluOpType.add)
                nc.vector.tensor_reduce(out=st1[:, 1:2], in_=s1p[:, NKC:2 * NKC],
                                        axis=mybir.AxisListType.X, op=mybir.AluOpType.add)
                nc.gpsimd.dma_start(out=bn1i_d[:], in_=st1[:])

            if stop >= 3:
                nc.gpsimd.collective_compute(
                    "AllReduce", mybir.AluOpType.add,
                    ins=[bn1i_d[:]], outs=[bn1o_d[:]], replica_groups=RG,
                )
                st1g = l1.tile([128, 2], F32)
                nc.gpsimd.dma_start(out=st1g[:], in_=bn1o_d[:])
                sc1, sh1 = _bn_apply_params(nc, l1, st1g, 0, N, g1_sb, be1_sb, "1")

                # apply BN1 + relu -> fp16, transpose to node-major, store
                h1T16 = l1.tile([H1, NB * 128], F16)
                for k in range(NKC):
                    nc.scalar.activation(
                        out=h1T16[:, k * 512 : (k + 1) * 512],
                        in_=h1T_sb[:, k * 512 : (k + 1) * 512],
                        func=mybir.ActivationFunctionType.Relu,
                        bias=sh1[:], scale=sc1[:],
                    )
                h1nm = l1.tile([128, NB, H1], F16)
                for t in range(NB):
                    pt = ps.tile([128, 128], F16, tag="pt")
                    nc.tensor.transpose(out=pt[:], in_=h1T16[:, t * 128 : (t + 1) * 128],
                                        identity=id16_sb[:])
                    nc.vector.tensor_copy(out=h1nm[:, t, :], in_=pt[:])
                nc.gpsimd.dma_start(
                    out=h1loc_d[:].rearrange("(t p) f -> p t f", p=128), in_=h1nm[:]
                )

            if stop >= 4:
                nc.gpsimd.collective_compute(
                    "AllGather", mybir.AluOpType.bypass,
                    ins=[h1loc_d[:]], outs=[h1full_d[:]], replica_groups=RG,
                )
            l1s.release()
            l1.release()

            # ---------- layer 2 ----------
            if stop >= 5:
                l2o = tc.alloc_tile_pool(name="l2o", bufs=1)
                l2a = tc.alloc_tile_pool(name="l2a", bufs=1)
                l2as = tc.alloc_tile_pool(name="l2as", bufs=3)
                agg2_sb = l2a.tile([128, NB, H1], F32)
                psum_map2 = {}
                for gcall in range(NGC):
                    hg = l2as.tile([128, CCH, H1], F16, tag="hg")
                    nc.gpsimd.dma_gather(
                        out_ap=hg[:],
                        in_ap=h1full_d[:],
                        idxs_ap=gidx_sb[:, gcall * (CPC // 16) : (gcall + 1) * (CPC // 16)],
                        num_idxs=CPC,
                        num_idxs_reg=CPC,
                        elem_size=H1,
                    )
                    for jj in range(CCH):
                        j = gcall * CCH + jj
                        b, cidx = divmod(j, C)
                        if cidx == 0:
                            acc2 = psacc.tile([128, H1], F32, tag="acc")
                            psum_map2[b] = acc2
                        nc.tensor.matmul(
                            out=psum_map2[b][:],
                            lhsT=oh[:, j, :],
                            rhs=hg[:, jj, :],
                            start=(cidx == 0),
                            stop=(cidx == C - 1),
                        )
                        if cidx == C - 1:
                            nc.vector.tensor_copy(out=agg2_sb[:, b, :], in_=psum_map2[b][:])

                # transpose agg2 -> [H1, PC]
                agghT = l2o.tile([H1, NB, 128], F32)
                for b in range(NB):
                    pt = ps.tile([H1, 128], F32, tag="pt")
                    nc.tensor.transpose(out=pt[:], in_=agg2_sb[:, b, :], identity=id32_sb[:])
                    nc.vector.tensor_copy(out=agghT[:, b, :], in_=pt[:])
                l2as.release()
                l2a.release()

                # project: h2T halves [128, PC] ; BN2 stats
                l2b = tc.alloc_tile_pool(name="l2b", bufs=1)
                l2bs = tc.alloc_tile_pool(name="l2bs", bufs=2)
                h2T_sb = l2b.tile([128, 2, NB * 128], F32)
                s2p = l2b.tile([128, 2, 16], F32)
                agghT_f = agghT[:].rearrange("p b n -> p (b n)")
                for half in range(2):
                    for k in range(NKC):
                        ph = ps.tile([128, 512], F32, tag="ph")
                        nc.tensor.matmul(
                            out=ph[:],
                            lhsT=w2_sb[:, half * 128 : (half + 1) * 128],
                            rhs=agghT_f[:, k * 512 : (k + 1) * 512],
                            start=True, stop=True,
                        )
                        nc.vector.tensor_copy(out=h2T_sb[:, half, k * 512 : (k + 1) * 512], in_=ph[:])
                        nc.vector.tensor_reduce(
                            out=s2p[:, half, k : k + 1], in_=ph[:],
                            axis=mybir.AxisListType.X, op=mybir.AluOpType.add,
                        )
                        sq = l2bs.tile([128, 512], F32, tag="sq2")
                        nc.scalar.square(out=sq[:], in_=ph[:])
                        nc.vector.tensor_reduce(
                            out=s2p[:, half, NKC + k : NKC + k + 1], in_=sq[:],
                            axis=mybir.AxisListType.X, op=mybir.AluOpType.add,
                        )
                st2 = l2b.tile([128, 4], F32)
                for half in range(2):
                    nc.vector.tensor_reduce(out=st2[:, 2 * half : 2 * half + 1],
                                            in_=s2p[:, half, 0:NKC],
                                            axis=mybir.AxisListType.X, op=mybir.AluOpType.add)
                    nc.vector.tensor_reduce(out=st2[:, 2 * half + 1 : 2 * half + 2],
                                            in_=s2p[:, half, NKC:2 * NKC],
                                            axis=mybir.AxisListType.X, op=mybir.AluOpType.add)
                nc.gpsimd.dma_start(out=bn2i_d[:], in_=st2[:])
                nc.gpsimd.collective_compute(
                    "AllReduce", mybir.AluOpType.add,
                    ins=[bn2i_d[:]], outs=[bn2o_d[:]], replica_groups=RG,
                )
                st2g = l2b.tile([128, 4], F32)
                nc.gpsimd.dma_start(out=st2g[:], in_=bn2o_d[:])

                h2T16 = l2b.tile([128, 2, NB * 128], F16)
                for half in range(2):
                    sc2, sh2 = _bn_apply_params(
                        nc, l2b, st2g, 2 * half, N,
                        g2_sb[:, half : half + 1], be2_sb[:, half : half + 1], "2",
                    )
                    for k in range(NKC):
                        nc.scalar.activation(
                            out=h2T16[:, half, k * 512 : (k + 1) * 512],
                            in_=h2T_sb[:, half, k * 512 : (k + 1) * 512],
                            func=mybir.ActivationFunctionType.Relu,
                            bias=sh2[:], scale=sc2[:],
                        )
                # transpose to node-major
                h2nm = l2o.tile([128, NB, 2, 128], F16)
                for t in range(NB):
                    for half in range(2):
                        pt = ps.tile([128, 128], F16, tag="pt")
                        nc.tensor.transpose(out=pt[:], in_=h2T16[:, half, t * 128 : (t + 1) * 128],
                                            identity=id16_sb[:])
                        nc.vector.tensor_copy(out=h2nm[:, t, half, :], in_=pt[:])
                l2bs.release()
                l2b.release()

            # ---------- mean-pool + head ----------
            if stop >= 6:
                tl = tc.alloc_tile_pool(name="tail", bufs=1)
                ohg = tl.tile([128, NB, G], F16)
                for t in range(NB):
                    nc.vector.tensor_scalar(
                        out=ohg[:, t, :],
                        in0=iota5_sb[:],
                        scalar1=bg_sb[:, t : t + 1],
                        scalar2=bw_sb[:, t : t + 1],
                        op0=mybir.AluOpType.is_equal,
                        op1=mybir.AluOpType.mult,
                    )
                poolT = tl.tile([128, 2, G], F32)
                for half in range(2):
                    pp = ps.tile([128, G], F32, tag="ph")
                    for t in range(NB):
                        nc.tensor.matmul(
                            out=pp[:],
                            lhsT=h2nm[:, t, half, :],
                            rhs=ohg[:, t, :],
                            start=(t == 0),
                            stop=(t == NB - 1),
                        )
                    nc.vector.tensor_copy(out=poolT[:, half, :], in_=pp[:])
                nc.gpsimd.dma_start(out=pli_d[:].rearrange("(h p) g -> p h g", p=128),
                                    in_=poolT[:])
                nc.gpsimd.collective_compute(
                    "AllReduce", mybir.AluOpType.add,
                    ins=[pli_d[:]], outs=[plo_d[:]], replica_groups=RG,
                )
                poolTg = tl.tile([128, 2, G], F32)
                nc.gpsimd.dma_start(out=poolTg[:],
                                    in_=plo_d[:].rearrange("(h p) g -> p h g", p=128))

                # head
                psv = ps.tile([128, G], F32, tag="ph")
                nc.tensor.matmul(out=psv[:], lhsT=ws_sb[:], rhs=sfT_sb[:],
                                 start=True, stop=True)
                solvT = tl.tile([128, G], F32)
                nc.scalar.activation(out=solvT[:], in_=psv[:],
                                     func=mybir.ActivationFunctionType.Relu,
                                     bias=bs_sb[:], scale=1.0)

                pzf = ps.tile([128, G], F32, tag="ph")
                zins = [poolTg[:, 0, :], poolTg[:, 1, :], solvT[:]]
                for k in range(3):
                    nc.tensor.matmul(
                        out=pzf[:], lhsT=wf1_sb[:, k, :], rhs=zins[k],
                        start=(k == 0), stop=(k == 2),
                    )
                zf_sb = tl.tile([128, G], F32)
                nc.vector.tensor_copy(out=zf_sb[:], in_=pzf[:])
                st3 = tl.tile([128, 2], F32)
                nc.vector.tensor_reduce(out=st3[:, 0:1], in_=zf_sb[:],
                                        axis=mybir.AxisListType.X, op=mybir.AluOpType.add)
                sq3 = tl.tile([128, G], F32)
                nc.scalar.square(out=sq3[:], in_=zf_sb[:])
                nc.vector.tensor_reduce(out=st3[:, 1:2], in_=sq3[:],
                                        axis=mybir.AxisListType.X, op=mybir.AluOpType.add)
                sc3, sh3 = _bn_apply_params(nc, tl, st3, 0, G, gf1_sb, bef1_sb, "3")
                zfa = tl.tile([128, G], F32)
                nc.scalar.activation(out=zfa[:], in_=zf_sb[:],
                                     func=mybir.ActivationFunctionType.Relu,
                                     bias=sh3[:], scale=sc3[:])

                po = ps.tile([1, G], F32, tag="ph")
                nc.tensor.matmul(out=po[:], lhsT=wf2_sb[:], rhs=zfa[:],
                                 start=True, stop=True)
                out_sb = tl.tile([1, G], F32)
                nc.vector.tensor_scalar(
                    out=out_sb[:], in0=po[:], scalar1=bf2_sb[:], scalar2=None,
                    op0=mybir.AluOpType.add,
                )
                nc.sync.dma_start(out=out_d[:].rearrange("n o -> o n"), in_=out_sb[:])
                tl.release()

            if stop >= 5:
                l2o.release()

    nc.finalize()
    _legalize_waits(nc)
    return nc


def _legalize_waits(nc, max_waits=1):
    """This walrus build rejects instructions with >1-2 sem waits. Hoist the
    excess onto preceding same-engine NoOps (sequencers run in program order)."""
    for fn in nc.m.functions:
        for bb in fn.blocks:
            new_insts = []
            for ins in bb.instructions:
                si = ins.sync_info
                if si is not None and si.on_wait and len(si.on_wait) > max_waits:
                    waits = list(si.on_wait)
                    keep = waits[: max_waits - 1] if max_waits > 1 else []
                    move = waits[len(keep):]
                    keep.append(move.pop())
                    for i, wv in enumerate(move):
                        nop = mybir.InstNoOp(name=f"{ins.name}_ws{i}", ins=[], outs=[],
                                             engine=ins.engine)
                        nop.sync_info = mybir.SyncInfo(on_wait=[wv], on_update=[])
                        new_insts.append(nop)
                        nc.register_instruction(nop, overwrite=True)
                    si.on_wait = keep
                new_insts.append(ins)
            bb.instructions[:] = new_insts


def _preprocess(x, edge_index, batch, solvent_fingerprint,
                W1, b1, g1, be1, W2, b2, g2, be2,
                Ws, bs, Wf1, bf1, gf1, bef1, Wf2, bf2):
    """Host-side sharding/index preprocessing. Returns (C, in_maps)."""
    edge_index = np.asarray(edge_index)
    batch = np.asarray(batch).astype(np.int64)
    x = np.ascontiguousarray(np.asarray(x, dtype=np.float32))

    loops = np.arange(N, dtype=np.int64)
    row = np.concatenate([edge_index[0].astype(np.int64), loops])
    col = np.concatenate([edge_index[1].astype(np.int64), loops])
    deg = np.bincount(col, minlength=N).astype(np.float32)
    dis = (1.0 / np.sqrt(deg)).astype(np.float32)
    norm = (dis[row] * dis[col]).astype(np.float32)

    perm = np.argsort(col, kind="stable")
    row, col, norm = row[perm], col[perm], norm[perm]

    gblk = col // 128
    nblk_tot = W * NB
    cnt = np.bincount(gblk, minlength=nblk_tot)
    C = int(np.ceil(cnt.max() / 128))
    NC = NB * C
    NE = NC * 128

    starts = np.zeros(nblk_tot + 1, np.int64)
    np.cumsum(cnt, out=starts[1:])
    rank = np.arange(row.shape[0]) - starts[gblk]

    ridx = np.zeros((W, NB, C * 128), np.int16)
    coll = np.full((W, NB, C * 128), 999.0, np.float32)
    nrm = np.zeros((W, NB, C * 128), np.float32)
    core_of = gblk // NB
    blk_of = gblk % NB
    ridx[core_of, blk_of, rank] = row.astype(np.int16)
    coll[core_of, blk_of, rank] = (col % 128).astype(np.float32)
    nrm[core_of, blk_of, rank] = norm

    gcnt = np.bincount(batch, minlength=G).astype(np.float32)
    node_g = np.full(NP, -1.0, np.float32)
    node_g[:N] = batch.astype(np.float32)
    node_w = np.zeros(NP, np.float32)
    node_w[:N] = 1.0 / np.maximum(gcnt, 1.0)[batch]

    rep = {
        "x": x,
        "iota128": np.tile(np.arange(128, dtype=np.float16)[None, :], (128, 1)),
        "iota512": np.tile(np.arange(G, dtype=np.float16)[None, :], (128, 1)),
        "ident16": np.eye(128, dtype=np.float16),
        "ident32": np.eye(128, dtype=np.float32),
        "sfT": np.ascontiguousarray(np.asarray(solvent_fingerprint, np.float32).T),
        "w1": np.asarray(W1, np.float32), "w2": np.asarray(W2, np.float32),
        "ws": np.asarray(Ws, np.float32),
        "wf1": np.ascontiguousarray(
            np.asarray(Wf1, np.float32).reshape(3, 128, 128).transpose(1, 0, 2)),
        "wf2": np.asarray(Wf2, np.float32).reshape(128, 1),
        "g1": np.asarray(g1, np.float32).reshape(128, 1),
        "be1": np.asarray(be1, np.float32).reshape(128, 1),
        "g2": np.ascontiguousarray(np.asarray(g2, np.float32).reshape(2, 128).T),
        "be2": np.ascontiguousarray(np.asarray(be2, np.float32).reshape(2, 128).T),
        "gf1": np.asarray(gf1, np.float32).reshape(128, 1),
        "bef1": np.asarray(bef1, np.float32).reshape(128, 1),
        "bs": np.asarray(bs, np.float32).reshape(128, 1),
        "bf2": np.asarray(bf2, np.float32).reshape(1, 1),
    }

    in_maps = []
    for c in range(W):
        r = ridx[c].reshape(NE)
        gidx = np.tile(r.reshape(NE // 16, 16).T, (8, 1)).copy()
        nodes = np.arange(c * PC, (c + 1) * PC)
        m = dict(rep)
        m.update({
            "gidx": gidx,
            "colloc": np.ascontiguousarray(coll[c].reshape(NC, 128).T),
            "enorm": np.ascontiguousarray(nrm[c].reshape(NC, 128).T),
            "batchg": np.ascontiguousarray(node_g[nodes].reshape(NB, 128).T),
            "bw": np.ascontiguousarray(node_w[nodes].reshape(NB, 128).T),
        })
        in_maps.append(m)
    return C, in_maps


_PROG_CACHE = {}


def _get_program(C, stop=99):
    key = (C, stop)
    if key not in _PROG_CACHE:
        _PROG_CACHE[key] = _build_program(C, stop)
    return _PROG_CACHE[key]


def kernel(**inputs) -> np.ndarray:
    C, in_maps = _preprocess(**inputs)
    nc = _get_program(C)
    res = run_bass_kernel_spmd(nc, in_maps, core_ids=list(range(W)))
    return np.asarray(res.results[0]["out"], dtype=np.float32)

